# revision 1
# baseline (speedup 1.0000x reference)
"""ExtAttention Trainium2 kernel (8 NeuronCores, SPMD).

Sharding: 8 cores = 4 batches x 2 query-row halves. Each core handles
batch b = core//2 and query rows [ih*1024, ih*1024+1024) with ih = core%2.
Softmax is over the key axis j (free dim), so row-sharding needs no
collectives; each core reads exactly its slice of the dominant `indicator`
tensor (42 MB/core) once.

Per-core dataflow (n=2048 keys, I=1024 query rows, H=4 heads, DH=32):
  - qkv projection on PE (q only for the local row half; scale folded into w_q)
  - per 32-row i-chunk and 512-col j-tile: one PSUM tile [(h,i32)=128, 512]
    accumulates three matmuls: sim (block-diag q stationary, K=(h,dh)=128),
    bias over indicator channels 0..3 (K=(c,i32)=128, sparse w_ind stationary),
    bias over channel 4 (K=32).
  - ACT exp PSUM->SBUF with accum_out giving row sums for free.
  - PE transpose (128x128 chunks) -> PSUM, DVE copy -> SBUF gives E^T.
  - av matmul: lhsT = v^T[j128,(h,d)], rhs = E^T[j128,(h,i32)], accumulated
    over all 16 j-chunks into one PSUM tile [(h,d),(h,i32)]; diagonal head
    blocks are the per-head attention outputs.
  - row-sum reciprocal is moved to the free dim with a DVE 32x32 block
    transpose; extraction of diag blocks fuses the 1/sum scaling (DVE STT).
  - output projection (w_out^T stationary) + bias, DMA out (256, 1024).
"""

import os
import sys

import numpy as np

for _p in ("/opt/trn_rl_repo", "/root/.axon_site/_ro/trn_rl_repo"):
    if os.path.isdir(_p) and _p not in sys.path:
        sys.path.insert(0, _p)

B, DIM, N, C, H, DH = 4, 256, 2048, 5, 4, 32
HID = H * DH            # 128
NCORES = 8
I = N // 2              # 1024 query rows per core
NIC = I // 32           # 32 i-chunks
NJT = N // 512          # 4 j-tiles
NJC = N // 128          # 16 j-chunks

_PROG = None
LAST_EXEC_NS = None
LAST_RESULTS = None


def _build_program(repeat=1):
    from contextlib import ExitStack

    import concourse.mybir as mybir
    import concourse.tile as tile
    from concourse import bacc
    from concourse.masks import make_identity

    f32 = mybir.dt.float32
    Alu = mybir.AluOpType
    Act = mybir.ActivationFunctionType

    nc = bacc.Bacc("TRN2", target_bir_lowering=False, debug=False,
                   num_devices=NCORES)

    x_d = nc.dram_tensor("x", [DIM, N], f32, kind="ExternalInput").ap()
    xq_d = nc.dram_tensor("xq", [DIM, I], f32, kind="ExternalInput").ap()
    ind_d = nc.dram_tensor("ind", [NIC, NJT, C * 32, 512], f32,
                           kind="ExternalInput").ap()
    wqkvT_d = nc.dram_tensor("wqkvT", [128, 2, 3 * HID], f32,
                             kind="ExternalInput").ap()
    s0_d = nc.dram_tensor("s0", [128, 128], f32, kind="ExternalInput").ap()
    s1_d = nc.dram_tensor("s1", [32, 128], f32, kind="ExternalInput").ap()
    woutT_d = nc.dram_tensor("woutT", [128, 2, 128], f32,
                             kind="ExternalInput").ap()
    bout_d = nc.dram_tensor("bout", [128, 2], f32, kind="ExternalInput").ap()
    out_d = nc.dram_tensor("out", [DIM, I], f32, kind="ExternalOutput").ap()

    with tile.TileContext(nc) as tc, ExitStack() as ctx:
        const = ctx.enter_context(tc.tile_pool(name="const", bufs=1))
        big = ctx.enter_context(tc.tile_pool(name="big", bufs=1))
        indp = ctx.enter_context(tc.tile_pool(name="indp", bufs=3))
        ep = ctx.enter_context(tc.tile_pool(name="ep", bufs=3))
        etp = ctx.enter_context(tc.tile_pool(name="etp", bufs=3))
        smallp = ctx.enter_context(tc.tile_pool(name="smallp", bufs=3))
        ps_mm = ctx.enter_context(tc.tile_pool(name="ps_mm", bufs=4, space="PSUM"))
        ps_t = ctx.enter_context(tc.tile_pool(name="ps_t", bufs=2, space="PSUM"))
        ps_av = ctx.enter_context(tc.tile_pool(name="ps_av", bufs=2, space="PSUM"))

        for _rep in range(repeat):
            # ---- constants ----
            wqkvT = const.tile([128, 2, 3 * HID], f32, tag="wqkvT")
            nc.sync.dma_start(wqkvT[:], wqkvT_d)
            s0 = const.tile([128, 128], f32, tag="s0")
            nc.sync.dma_start(s0[:], s0_d)
            s1 = const.tile([32, 128], f32, tag="s1")
            nc.sync.dma_start(s1[:], s1_d)
            woutT = const.tile([128, 2, 128], f32, tag="woutT")
            nc.sync.dma_start(woutT[:], woutT_d)
            bout = const.tile([128, 2], f32, tag="bout")
            nc.sync.dma_start(bout[:], bout_d)
            ident = const.tile([128, 128], f32, tag="ident")
            make_identity(nc, ident[:])

            # ---- load x ----
            x_sb = big.tile([128, 2, N], f32, tag="x_sb")
            xq_sb = big.tile([128, 2, I], f32, tag="xq_sb")
            for kc in range(2):
                nc.sync.dma_start(x_sb[:, kc, :], x_d[kc * 128:(kc + 1) * 128, :])
                nc.sync.dma_start(xq_sb[:, kc, :], xq_d[kc * 128:(kc + 1) * 128, :])

            # ---- qkv projection ----
            q_sb = big.tile([128, I], f32, tag="q_sb")     # [(h,d), i_local]  (scale folded)
            k_sb = big.tile([128, N], f32, tag="k_sb")     # [(h,d), j]
            v_sb = big.tile([128, N], f32, tag="v_sb")     # [(h,d), j]
            vT_sb = big.tile([128, NJC, 128], f32, tag="vT_sb")  # [j128, jc, (h,d)]

            for nt in range(I // 512):
                ps = ps_mm.tile([128, 512], f32, tag="mm")
                for kc in range(2):
                    nc.tensor.matmul(ps[:], wqkvT[:, kc, 0:128],
                                     xq_sb[:, kc, nt * 512:(nt + 1) * 512],
                                     start=(kc == 0), stop=(kc == 1))
                nc.scalar.copy(q_sb[:, nt * 512:(nt + 1) * 512], ps[:])
            for dst, lo in ((k_sb, 128), (v_sb, 256)):
                for nt in range(N // 512):
                    ps = ps_mm.tile([128, 512], f32, tag="mm")
                    for kc in range(2):
                        nc.tensor.matmul(ps[:], wqkvT[:, kc, lo:lo + 128],
                                         x_sb[:, kc, nt * 512:(nt + 1) * 512],
                                         start=(kc == 0), stop=(kc == 1))
                    nc.scalar.copy(dst[:, nt * 512:(nt + 1) * 512], ps[:])

            # ---- v transpose: vT[j128, (h,d)] per j-chunk ----
            for jc in range(NJC):
                pst = ps_t.tile([128, 512], f32, tag="pst")
                nc.tensor.transpose(pst[:, 0:128],
                                    v_sb[:, jc * 128:(jc + 1) * 128], ident[:])
                nc.vector.tensor_copy(vT_sb[:, jc, :], pst[:, 0:128])

            # ---- block-diag q stationary for all i-chunks ----
            qbd = big.tile([128, NIC, 128], f32, tag="qbd")
            nc.any.memset(qbd[:], 0.0)
            for h in range(H):
                nc.vector.tensor_copy(
                    qbd[h * 32:(h + 1) * 32, :, h * 32:(h + 1) * 32],
                    q_sb[h * 32:(h + 1) * 32, :].rearrange(
                        "p (ic w) -> p ic w", w=32),
                )

            hidden = big.tile([128, I], f32, tag="hidden")

            # ---- main attention loop ----
            # Per 32-row i-chunk: 4 j-tiles of [(h,i32)=128, 512] sim+bias
            # PSUM accumulation -> exp (row sums via accum_out) -> PE
            # transpose per 128-chunk -> DVE drain -> N=128 av matmuls
            # accumulating [(h,d), (h,i32)] over all 16 j-chunks.
            for ic in range(NIC):
                isl = slice(ic * 32, (ic + 1) * 32)
                rs4 = smallp.tile([128, 4], f32, tag="rs4", name="rs4")
                av = ps_av.tile([128, 128], f32, tag="av", name="av")
                for jt in range(NJT):
                    jsl = slice(jt * 512, (jt + 1) * 512)
                    indA = indp.tile([128, 512], f32, tag="indA", name="indA")
                    nc.sync.dma_start(indA[:], ind_d[ic, jt, 0:128, :])
                    indB = indp.tile([32, 512], f32, tag="indB", name="indB")
                    nc.sync.dma_start(indB[:], ind_d[ic, jt, 128:160, :])

                    ps = ps_mm.tile([128, 512], f32, tag="mm", name="ps")
                    nc.tensor.matmul(ps[:], qbd[:, ic, :], k_sb[:, jsl],
                                     start=True, stop=False)
                    nc.tensor.matmul(ps[:], s0[:], indA[:],
                                     start=False, stop=False)
                    nc.tensor.matmul(ps[:], s1[:], indB[:],
                                     start=False, stop=True)

                    e = ep.tile([128, 512], f32, tag="e", name="e")
                    nc.scalar.activation(e[:], ps[:], Act.Exp,
                                         accum_out=rs4[:, jt:jt + 1])

                    pst = ps_t.tile([128, 512], f32, tag="pst", name="pst")
                    for kc in range(4):
                        nc.tensor.transpose(pst[:, kc * 128:(kc + 1) * 128],
                                            e[:, kc * 128:(kc + 1) * 128],
                                            ident[:])
                    et = etp.tile([128, 512], f32, tag="et", name="et")
                    nc.vector.tensor_copy(et[:], pst[:])

                    for kc in range(4):
                        jc = jt * 4 + kc
                        nc.tensor.matmul(av[:], vT_sb[:, jc, :],
                                         et[:, kc * 128:(kc + 1) * 128],
                                         start=(jc == 0), stop=(jc == NJC - 1),
                                         skip_group_check=True)

                # row sums -> reciprocal -> free dim (DVE 32x32 transpose)
                rs1 = smallp.tile([128, 1], f32, tag="rs1", name="rs1")
                nc.vector.tensor_reduce(rs1[:], rs4[:],
                                        axis=mybir.AxisListType.X, op=Alu.add)
                recip32 = smallp.tile([128, 32], f32, tag="recip32",
                                      name="recip32")
                nc.vector.reciprocal(recip32[:], rs1[:].to_broadcast((128, 32)))
                rsT = smallp.tile([128, 32], f32, tag="rsT", name="rsT")
                nc.vector.transpose(rsT[:], recip32[:])
                # rsT[32h+y, i'] = 1/rowsum(h, i') for every y

                for h in range(H):
                    hsl = slice(h * 32, (h + 1) * 32)
                    nc.vector.scalar_tensor_tensor(
                        out=hidden[hsl, isl],
                        in0=av[hsl, hsl],
                        scalar=1.0,
                        in1=rsT[hsl, 0:32],
                        op0=Alu.mult,
                        op1=Alu.mult,
                    )

            # ---- output projection ----
            for oc in range(2):
                for it in range(2):
                    ps = ps_mm.tile([128, 512], f32, tag="mm")
                    nc.tensor.matmul(ps[:], woutT[:, oc, :],
                                     hidden[:, it * 512:(it + 1) * 512],
                                     start=True, stop=True)
                    osb = smallp.tile([128, 512], f32, tag="osb")
                    nc.scalar.add(osb[:], ps[:], bout[:, oc:oc + 1])
                    nc.sync.dma_start(
                        out_d[oc * 128:(oc + 1) * 128, it * 512:(it + 1) * 512],
                        osb[:])

    nc.compile()
    return nc


def _host_prep(w_qkv, w_ind, w_out, b_out):
    wqkv_s = np.ascontiguousarray(w_qkv, dtype=np.float32).copy()
    wqkv_s[:HID] *= np.float32(DH ** -0.5)
    wqkvT = np.ascontiguousarray(wqkv_s.T)          # (256, 384)
    wqkvT = np.ascontiguousarray(
        wqkvT.reshape(2, 128, 3 * HID))              # (2,128,384)
    wqkvT = np.ascontiguousarray(wqkvT.transpose(1, 0, 2))  # (128,2,384)

    S0 = np.zeros((128, 128), np.float32)
    S1 = np.zeros((32, 128), np.float32)
    ii = np.arange(32)
    for h in range(H):
        for c in range(4):
            S0[c * 32 + ii, h * 32 + ii] = w_ind[h, c]
        S1[ii, h * 32 + ii] = w_ind[h, 4]

    woutT = np.ascontiguousarray(w_out.T.astype(np.float32))     # (128, 256)
    woutT = np.ascontiguousarray(
        woutT.reshape(128, 2, 128))                  # (128,2,128)
    bout = np.ascontiguousarray(
        b_out.astype(np.float32).reshape(2, 128).T)  # (128,2)
    return wqkvT, S0, S1, woutT, bout


def _tile_ind(ind):
    """(C, I, N) -> (NIC, NJT, C*32, 512) with each tile contiguous.

    Row c*32+i of tile (ic, jt) = ind[c, ic*32+i, jt*512:(jt+1)*512], the
    exact (c, i32) partition layout the S0/S1 bias stationaries expect.
    """
    t = ind.reshape(C, NIC, 32, NJT, 512).transpose(1, 3, 0, 2, 4)
    return np.ascontiguousarray(t.reshape(NIC, NJT, C * 32, 512))


def kernel(x, indicator, w_qkv, w_ind, w_out, b_out):
    global _PROG
    from concourse.bass_utils import run_bass_kernel_spmd

    if _PROG is None:
        _PROG = _build_program()
    nc = _PROG

    x = np.ascontiguousarray(np.asarray(x, dtype=np.float32))
    indicator = np.asarray(indicator, dtype=np.float32)
    wqkvT, S0, S1, woutT, bout = _host_prep(
        np.asarray(w_qkv), np.asarray(w_ind), np.asarray(w_out),
        np.asarray(b_out))

    in_maps = []
    for core in range(NCORES):
        b, ih = core // 2, core % 2
        i0 = ih * I
        in_maps.append({
            "x": x[b],
            "xq": np.ascontiguousarray(x[b][:, i0:i0 + I]),
            "ind": _tile_ind(indicator[b, :, i0:i0 + I, :]),
            "wqkvT": wqkvT,
            "s0": S0,
            "s1": S1,
            "woutT": woutT,
            "bout": bout,
        })

    trace = os.environ.get("EXT_ATTN_TRACE") == "1"
    res = run_bass_kernel_spmd(nc, in_maps, list(range(NCORES)), trace=trace)
    global LAST_EXEC_NS, LAST_RESULTS
    LAST_EXEC_NS = res.exec_time_ns
    LAST_RESULTS = res
    out = np.empty((B, DIM, N), np.float32)
    for core in range(NCORES):
        b, ih = core // 2, core % 2
        out[b, :, ih * I:(ih + 1) * I] = res.results[core]["out"]
    return out


if __name__ == "__main__":
    rng = np.random.default_rng(0)
    ins = {
        "x": rng.standard_normal((B, DIM, N), dtype=np.float32),
        "indicator": rng.standard_normal((B, C, N, N), dtype=np.float32),
        "w_qkv": rng.standard_normal((3 * HID, DIM), dtype=np.float32) * DIM ** -0.5,
        "w_ind": rng.standard_normal((H, C), dtype=np.float32) * C ** -0.5,
        "w_out": rng.standard_normal((DIM, HID), dtype=np.float32) * HID ** -0.5,
        "b_out": np.zeros((DIM,), np.float32),
    }
    out = kernel(**ins)
    print("kernel ran, out shape", out.shape, "mean", float(np.abs(out).mean()))



# revision 6
# speedup vs baseline: 3.7612x; 3.7612x over previous
"""ExtAttention Trainium2 kernel (8 NeuronCores, SPMD), v2: bf16 + transposed scores.

Sharding: 8 cores = 4 batches x 2 query-row halves (core = 2*b + ih; rows
[ih*1024, ih*1024+1024)). Softmax is over keys j, so row-sharding needs no
collectives; each core streams its 21 MB bf16 slice of `indicator` once.

Per-core dataflow (N=2048 keys, I=1024 query rows, H=4 heads, DH=32):
  Scores are computed TRANSPOSED: [j (partitions), (icq, h, i16) (free)] so
  the exp output E^T is directly consumable by the A@V matmul with v^T
  stationary -- no PE transposes of E in the main loop.
  - i-chunks of 16 rows so all 5 indicator channels fit one K=80 contraction:
    one bias matmul per (ic, jc) instead of two (channels 0..3 + channel 4).
  - per (group of 8 i-chunks, j-chunk): 16 matmuls build a [j128, 512] PSUM
    tile (sim via k-chunk stationary x block-diag q + bias via ind-tile
    stationary x sparse S), ACT exp -> E^T bf16 in SBUF.
  - av: vT_jc stationary, E^T moving, accumulated over 16 jc into one PSUM
    bank [(h,d), 512]. Row sums: 1-column matmuls (E^T pair-slice stationary,
    ones moving) accumulated into a persistent [128, 32] PSUM tile.
  - normalization: reciprocal + DVE 32x32 block transpose puts 1/rowsum on
    the (h,d) partition axis; scalar_tensor_tensor fuses the scaling with
    extraction into hidden[(h,d), i] bf16.
  - qkv projection, v transpose, and output projection as in v1 but bf16.

All matmuls are bf16 (4x PE throughput vs fp32); PSUM accumulation stays f32.
The indicator tensor is cast to bf16 host-side, which also halves DMA bytes.
"""

import os
import sys

import numpy as np

for _p in ("/opt/trn_rl_repo", "/root/.axon_site/_ro/trn_rl_repo"):
    if os.path.isdir(_p) and _p not in sys.path:
        sys.path.insert(0, _p)

B, DIM, N, C, H, DH = 4, 256, 2048, 5, 4, 32
HID = H * DH            # 128
NCORES = 8
I = N // 2              # 1024 query rows per core
IC = 16                 # i-chunk size (5 channels * 16 = 80 <= 128)
NIC = I // IC           # 64 i-chunks
NG = 8                  # groups of 8 i-chunks -> 512 PSUM columns
NJC = N // 128          # 16 j-chunks

_PROG = None
LAST_EXEC_NS = None
LAST_RESULTS = None


def _build_program():
    from contextlib import ExitStack

    import concourse.mybir as mybir
    import concourse.tile as tile
    from concourse import bacc
    from concourse.masks import make_identity

    f32 = mybir.dt.float32
    bf16 = mybir.dt.bfloat16
    Alu = mybir.AluOpType
    Act = mybir.ActivationFunctionType

    nc = bacc.Bacc("TRN2", target_bir_lowering=False, debug=False,
                   num_devices=NCORES)

    xt_d = nc.dram_tensor("xt", [128, 2, N], bf16, kind="ExternalInput").ap()
    xq_d = nc.dram_tensor("xq", [128, 2, I], bf16, kind="ExternalInput").ap()
    # per group g: rows (c, i16), cols (icq8, j2048)
    ind_d = nc.dram_tensor("ind", [NG, C * IC, 8, N], bf16,
                           kind="ExternalInput").ap()
    wqkvT_d = nc.dram_tensor("wqkvT", [128, 2, 3 * HID], bf16,
                             kind="ExternalInput").ap()
    s_d = nc.dram_tensor("s", [C * IC, 64], bf16, kind="ExternalInput").ap()
    woutT_d = nc.dram_tensor("woutT", [128, 2, 128], bf16,
                             kind="ExternalInput").ap()
    bout_d = nc.dram_tensor("bout", [128, 2], f32, kind="ExternalInput").ap()
    out_d = nc.dram_tensor("out", [DIM, I], f32, kind="ExternalOutput").ap()

    with tile.TileContext(nc) as tc, ExitStack() as ctx:
        const = ctx.enter_context(tc.tile_pool(name="const", bufs=1))
        big = ctx.enter_context(tc.tile_pool(name="big", bufs=1))
        indp = ctx.enter_context(tc.tile_pool(name="indp", bufs=2))
        etp = ctx.enter_context(tc.tile_pool(name="etp", bufs=3))
        smallp = ctx.enter_context(tc.tile_pool(name="smallp", bufs=4))
        ps_mm = ctx.enter_context(tc.tile_pool(name="ps_mm", bufs=3, space="PSUM"))
        ps_av = ctx.enter_context(tc.tile_pool(name="ps_av", bufs=2, space="PSUM"))
        ps_z = ctx.enter_context(tc.tile_pool(name="ps_z", bufs=1, space="PSUM"))
        ps_t = ctx.enter_context(tc.tile_pool(name="ps_t", bufs=2, space="PSUM"))

        # ---- constants ----
        wqkvT = const.tile([128, 2, 3 * HID], bf16, tag="wqkvT")
        nc.sync.dma_start(wqkvT[:], wqkvT_d)
        s_sb = const.tile([C * IC, 64], bf16, tag="s_sb")
        nc.sync.dma_start(s_sb[:], s_d)
        woutT = const.tile([128, 2, 128], bf16, tag="woutT")
        nc.sync.dma_start(woutT[:], woutT_d)
        bout = const.tile([128, 2], f32, tag="bout")
        nc.sync.dma_start(bout[:], bout_d)
        ident = const.tile([128, 128], f32, tag="ident")
        make_identity(nc, ident[:])
        ones = const.tile([128, 1], bf16, tag="ones")
        nc.any.memset(ones[:], 1.0)

        # ---- load x ----
        x_sb = big.tile([128, 2, N], bf16, tag="x_sb")
        nc.sync.dma_start(x_sb[:], xt_d)
        xq_sb = big.tile([128, 2, I], bf16, tag="xq_sb")
        nc.sync.dma_start(xq_sb[:], xq_d)

        # ---- qkv projection (bf16 out; q has scale folded in weights) ----
        q_sb = big.tile([128, I], bf16, tag="q_sb")      # [(h,d), i_local]
        k_sb = big.tile([128, N], bf16, tag="k_sb")      # [(h,d), j]
        v_f32 = big.tile([128, N], f32, tag="v_f32")     # [(h,d), j]
        vT_sb = big.tile([128, NJC, 128], bf16, tag="vT_sb")  # [j128, jc, (h,d)]

        for nt in range(I // 512):
            ps = ps_t.tile([128, 512], f32, tag="pst")
            for kc in range(2):
                nc.tensor.matmul(ps[:], wqkvT[:, kc, 0:128],
                                 xq_sb[:, kc, nt * 512:(nt + 1) * 512],
                                 start=(kc == 0), stop=(kc == 1))
            nc.scalar.copy(q_sb[:, nt * 512:(nt + 1) * 512], ps[:])
        for dst, lo in ((k_sb, 128), (v_f32, 256)):
            for nt in range(N // 512):
                ps = ps_t.tile([128, 512], f32, tag="pst")
                for kc in range(2):
                    nc.tensor.matmul(ps[:], wqkvT[:, kc, lo:lo + 128],
                                     x_sb[:, kc, nt * 512:(nt + 1) * 512],
                                     start=(kc == 0), stop=(kc == 1))
                nc.scalar.copy(dst[:, nt * 512:(nt + 1) * 512], ps[:])

        # ---- v transpose: vT[j128, (h,d)] per j-chunk (f32 PE transpose,
        # bf16 cast on the DVE drain) ----
        for jc in range(NJC):
            pst = ps_t.tile([128, 512], f32, tag="pst")
            nc.tensor.transpose(pst[:, 0:128],
                                v_f32[:, jc * 128:(jc + 1) * 128], ident[:])
            nc.vector.tensor_copy(vT_sb[:, jc, :], pst[:, 0:128])

        # ---- block-diag q, rhs for the sim matmul: [(h,d), ic, (h,i16)] ----
        qbd = big.tile([128, NIC, 64], bf16, tag="qbd")
        nc.any.memset(qbd[:], 0.0)
        for h in range(H):
            nc.vector.tensor_copy(
                qbd[h * 32:(h + 1) * 32, :, h * IC:(h + 1) * IC],
                q_sb[h * 32:(h + 1) * 32, :].rearrange(
                    "p (ic w) -> p ic w", w=IC),
            )

        hidden = big.tile([128, I], bf16, tag="hidden")
        zps = ps_z.tile([128, 32], f32, tag="zps")  # rowsums, col = ic-pair

        # ---- main attention loop ----
        for g in range(NG):
            ind_t = indp.tile([C * IC, 8, N], bf16, tag="ind", name="ind")
            nc.sync.dma_start(ind_t[:], ind_d[g])
            av = ps_av.tile([128, 512], f32, tag="av", name="av")
            for jc in range(NJC):
                jsl = slice(jc * 128, (jc + 1) * 128)
                ps = ps_mm.tile([128, 512], f32, tag="mm", name="ps")
                # One accumulation group per PSUM bank: start=True marks the
                # whole 2KB zero region pending, so each slice's first write
                # overwrites (lazy zero) and its second accumulates.
                for icq in range(8):
                    ic = g * 8 + icq
                    osl = slice(icq * 64, (icq + 1) * 64)
                    nc.tensor.matmul(ps[:, osl], k_sb[:, jsl],
                                     qbd[:, ic, :], start=(icq == 0),
                                     stop=False)
                    nc.tensor.matmul(ps[:, osl], ind_t[:, icq, jsl],
                                     s_sb[:], start=False, stop=(icq == 7))
                et = etp.tile([128, 512], bf16, tag="et", name="et")
                nc.scalar.activation(et[:], ps[:], Act.Exp)
                nc.tensor.matmul(av[:], vT_sb[:, jc, :], et[:],
                                 start=(jc == 0), stop=(jc == NJC - 1),
                                 skip_group_check=True)
                for p2 in range(4):
                    pair = g * 4 + p2
                    # zps partitions follow et cols: (ic2, h, i16).
                    # Single accumulation group for the whole kernel: the
                    # one start marks the bank pending once; each column's
                    # first write then overwrites, later ones accumulate.
                    # (A start per pair would re-mark the whole bank and
                    # drop sibling columns' already-accumulated values.)
                    nc.tensor.matmul(zps[:, pair:pair + 1],
                                     et[:, p2 * 128:(p2 + 1) * 128], ones[:],
                                     start=(g == 0 and jc == 0 and p2 == 0),
                                     stop=(g == NG - 1 and jc == NJC - 1
                                           and p2 == 3),
                                     skip_group_check=True)

            # ---- normalize + extract hidden[(h,d), i] ----
            for p2 in range(4):
                pair = g * 4 + p2
                recipb = smallp.tile([128, 32], f32, tag="recipb",
                                     name="recipb")
                nc.vector.reciprocal(
                    recipb[:], zps[:, pair:pair + 1].to_broadcast((128, 32)))
                rsT = smallp.tile([128, 32], f32, tag="rsT", name="rsT")
                nc.vector.transpose(rsT[:], recipb[:])
                # zps partitions are (ic2, h, i16); after the 32x32 block
                # transpose, rsT[32b + y, (h%2)*16 + i'] = 1/Z[ic', h, i']
                # for every y, where b = ic'*2 + h//2.
                for ic2 in range(2):
                    icq = p2 * 2 + ic2
                    ic = g * 8 + icq
                    for h in range(H):
                        hsl = slice(h * 32, (h + 1) * 32)
                        bb = ic2 * 2 + h // 2
                        nc.vector.scalar_tensor_tensor(
                            out=hidden[hsl, ic * IC:(ic + 1) * IC],
                            in0=av[hsl, icq * 64 + h * IC:
                                   icq * 64 + (h + 1) * IC],
                            scalar=1.0,
                            in1=rsT[bb * 32:(bb + 1) * 32,
                                    (h % 2) * IC:(h % 2 + 1) * IC],
                            op0=Alu.mult,
                            op1=Alu.mult,
                        )

        # ---- output projection ----
        for oc in range(2):
            for it in range(2):
                ps = ps_t.tile([128, 512], f32, tag="pst")
                nc.tensor.matmul(ps[:], woutT[:, oc, :],
                                 hidden[:, it * 512:(it + 1) * 512],
                                 start=True, stop=True)
                osb = smallp.tile([128, 512], f32, tag="osb")
                nc.scalar.add(osb[:], ps[:], bout[:, oc:oc + 1])
                nc.sync.dma_start(
                    out_d[oc * 128:(oc + 1) * 128, it * 512:(it + 1) * 512],
                    osb[:])

    nc.compile()
    return nc


def _host_prep(w_qkv, w_ind, w_out, b_out):
    import ml_dtypes

    bf16 = ml_dtypes.bfloat16

    wqkv_s = np.ascontiguousarray(w_qkv, dtype=np.float32).copy()
    wqkv_s[:HID] *= np.float32(DH ** -0.5)
    wqkvT = np.ascontiguousarray(wqkv_s.T)               # (256, 384)
    wqkvT = wqkvT.reshape(2, 128, 3 * HID).transpose(1, 0, 2)  # (128,2,384)
    wqkvT = np.ascontiguousarray(wqkvT).astype(bf16)

    # S[(c,i2), (h,i1)] = w_ind[h, c] * (i1 == i2)
    S = np.zeros((C * IC, 64), np.float32)
    ii = np.arange(IC)
    for h in range(H):
        for c in range(C):
            S[c * IC + ii, h * IC + ii] = w_ind[h, c]
    S = S.astype(bf16)

    woutT = np.ascontiguousarray(w_out.T.astype(np.float32))     # (128, 256)
    woutT = np.ascontiguousarray(
        woutT.reshape(128, 2, 128)).astype(bf16)         # (128,2,128)
    bout = np.ascontiguousarray(
        b_out.astype(np.float32).reshape(2, 128).T)      # (128,2)
    return wqkvT, S, woutT, bout


def _tile_ind(ind, bf16):
    """(C, I, N) f32 -> (NG, C*16, 8, N) bf16.

    Row c*16+i2 of group g, slot icq = ind[c, (g*8+icq)*16 + i2, :].
    """
    t = ind.reshape(C, NG, 8, IC, N).transpose(1, 0, 3, 2, 4)
    return np.ascontiguousarray(t.reshape(NG, C * IC, 8, N)).astype(bf16)


def kernel(x, indicator, w_qkv, w_ind, w_out, b_out):
    global _PROG
    import ml_dtypes
    from concourse.bass_utils import run_bass_kernel_spmd

    bf16 = ml_dtypes.bfloat16

    if _PROG is None:
        _PROG = _build_program()
    nc = _PROG

    x = np.ascontiguousarray(np.asarray(x, dtype=np.float32))
    indicator = np.asarray(indicator, dtype=np.float32)
    wqkvT, S, woutT, bout = _host_prep(
        np.asarray(w_qkv), np.asarray(w_ind), np.asarray(w_out),
        np.asarray(b_out))

    in_maps = []
    for core in range(NCORES):
        b, ih = core // 2, core % 2
        i0 = ih * I
        xb = x[b].reshape(2, 128, N).transpose(1, 0, 2)  # (128, 2, N)
        in_maps.append({
            "xt": np.ascontiguousarray(xb).astype(bf16),
            "xq": np.ascontiguousarray(xb[:, :, i0:i0 + I]).astype(bf16),
            "ind": _tile_ind(indicator[b, :, i0:i0 + I, :], bf16),
            "wqkvT": wqkvT,
            "s": S,
            "woutT": woutT,
            "bout": bout,
        })

    trace = os.environ.get("EXT_ATTN_TRACE") == "1"
    res = run_bass_kernel_spmd(nc, in_maps, list(range(NCORES)), trace=trace)
    global LAST_EXEC_NS, LAST_RESULTS
    LAST_EXEC_NS = res.exec_time_ns
    LAST_RESULTS = res
    out = np.empty((B, DIM, N), np.float32)
    for core in range(NCORES):
        b, ih = core // 2, core % 2
        out[b, :, ih * I:(ih + 1) * I] = res.results[core]["out"]
    return out


if __name__ == "__main__":
    rng = np.random.default_rng(0)
    ins = {
        "x": rng.standard_normal((B, DIM, N), dtype=np.float32),
        "indicator": rng.standard_normal((B, C, N, N), dtype=np.float32),
        "w_qkv": rng.standard_normal((3 * HID, DIM), dtype=np.float32) * DIM ** -0.5,
        "w_ind": rng.standard_normal((H, C), dtype=np.float32) * C ** -0.5,
        "w_out": rng.standard_normal((DIM, HID), dtype=np.float32) * HID ** -0.5,
        "b_out": np.zeros((DIM,), np.float32),
    }
    out = kernel(**ins)
    print("kernel ran, out shape", out.shape, "mean", float(np.abs(out).mean()))
